# revision 9
# baseline (speedup 1.0000x reference)
"""Depthwise 4x4 blur (upfirdn2d pad=(2,1)) on TRN2, 8 NeuronCores.

Math: out[h,w] = sum_{i,j} Kf[i,j] * x[h+i-2, w+j-2]   (Kf = flipped 2D kernel,
out-of-range terms = zero padding). For each kernel column j this is a banded
128x128 matrix A_j applied over H to a W-shifted slice of the padded image:

    OUT = sum_j A_j @ Xpad[:, j:j+128]      (PSUM accumulation over j)

H-padding is folded into the band clipping of A_j; W-padding is baked into the
host-side layout (stride-131 rows: [0, 0, x0..x127, 0]). Sharding: batch dim
(8 batches -> 8 cores), each core processes 256 images of 128x128.

bf16 everywhere (tolerance is 2e-2; measured rel err ~4e-3): halves HBM
traffic vs f32 (HBM floor ~47 us/core) and streams the PE at 1 col/cycle.
Host-side the block is transposed to [H, C, WP] / [H, C, W] so every DMA is a
dense 2D pattern (multi-KB contiguous run per partition).

v3: the blur is separable, so A_j = kw[j] * Ab. Taps pair up:

    kw0*Ab@Xp0 + kw1*Ab@Xp1 = A_0 @ (Xp0 + (kw1/kw0)*Xp1) = A_0 @ u
    kw3*Ab@Xp3 + kw2*Ab@Xp2 = A_3 @ (Xp3 + (kw2/kw3)*Xp2) = A_3 @ v

u/v are ONE fused DVE op each (scalar_tensor_tensor), so most image groups
need only 2 matmuls instead of 4, moving half the PE work to the otherwise
idle Vector engine. Every MM4_EVERY-th group keeps the plain 4-matmul path to
balance PE (~35us) against DVE (~32us). PSUM->SBUF copies alternate between
Scalar and GpSimd. All engines then sit below the ~47.5us DMA floor.
"""

import numpy as np
from contextlib import ExitStack

import concourse.bass as bass
import concourse.bacc as bacc
import concourse.tile as tile
import concourse.mybir as mybir
from concourse.bass_utils import run_bass_kernel_spmd

N_CORES = 8
B, C, H, W = 8, 256, 128, 128
WP = W + 3         # padded image stride: [0, 0, x0..x127, 0]
GROUP = 4          # images per PSUM bank (4*128 = 512 f32 = one bank)
SUPER = 32         # images per DMA (~8 KB contiguous per partition)
MM4_EVERY = 4      # every Nth group uses the 4-matmul path (PE/DVE balance)

F32 = mybir.dt.float32
BF16 = mybir.dt.bfloat16
MULT = mybir.AluOpType.mult
ADD = mybir.AluOpType.add


def _body(ctx, tc, o_ap, x_ap, w_ap, s_u, s_v):
    nc = tc.nc
    wpool = ctx.enter_context(tc.tile_pool(name="wts", bufs=1))
    xpool = ctx.enter_context(tc.tile_pool(name="xin", bufs=4))
    upool = ctx.enter_context(tc.tile_pool(name="uv", bufs=12))
    opool = ctx.enter_context(tc.tile_pool(name="oup", bufs=4))
    ppool = ctx.enter_context(tc.tile_pool(name="ps", bufs=8, space="PSUM"))

    wt = wpool.tile([H, 4 * H], BF16)

    # ramp-up / ramp-down supertile sizes: small tiles at the ends prime and
    # drain the DMA->prep->matmul->copy->DMA pipeline faster
    sizes = [2, 2, 4, 8, 16] + [SUPER] * ((C - 64) // SUPER) + [16, 8, 4, 2, 2]
    assert sum(sizes) == C

    # first data tile + weights lead on the ACT ring; steady-state input DMA
    # owns the SP ring while outputs go back on ACT
    first = True
    c0 = 0
    gidx = 0
    for sz in sizes:
        xt = xpool.tile([H, sz * WP], BF16, tag="xt")
        xt3 = xt[:].rearrange("h (c w) -> h c w", c=sz)
        eng = nc.scalar if first else nc.sync
        eng.dma_start(xt3, x_ap[:, c0 : c0 + sz])
        if first:
            nc.scalar.dma_start(wt[:], w_ap)
            first = False
        ot = opool.tile([H, sz * W], BF16, tag="ot")
        groups = [(i * GROUP, min(GROUP, sz - i * GROUP))
                  for i in range((sz + GROUP - 1) // GROUP)]
        for gi, gc in groups:
            pt = ppool.tile([H, gc * W], F32, tag="pt")
            if gidx % MM4_EVERY == 0:
                for j in range(4):
                    nc.tensor.matmul(
                        pt[:], wt[:, j * H : (j + 1) * H],
                        xt3[:, gi : gi + gc, j : j + W],
                        start=(j == 0), stop=(j == 3),
                    )
            else:
                # GpSimd can't touch PSUM and only has plain tensor_tensor,
                # but the SBUF->SBUF prep adds are exactly that (when the
                # kernel is symmetric); it takes every 4th prep group off
                # the Vector engine
                peng = nc.gpsimd if gidx % MM4_EVERY == 2 else nc.vector
                uv = upool.tile([H, 2 * gc * W], BF16, tag="uv")
                uv3 = uv[:].rearrange("h (k c w) -> h k c w", k=2, c=gc)
                for k, (ja, jb, s) in enumerate(((0, 3, s_u), (1, 2, s_v))):
                    va = xt3[:, gi : gi + gc, ja : ja + W]
                    vb = xt3[:, gi : gi + gc, jb : jb + W]
                    if s == 1.0:
                        peng.tensor_tensor(uv3[:, k], va, vb, ADD)
                    else:
                        nc.vector.scalar_tensor_tensor(
                            uv3[:, k], vb, s, va, MULT, ADD
                        )
                nc.tensor.matmul(pt[:], wt[:, :H], uv3[:, 0], start=True, stop=False)
                nc.tensor.matmul(pt[:], wt[:, H : 2 * H], uv3[:, 1], start=False, stop=True)
            nc.scalar.copy(ot[:, gi * W : (gi + gc) * W], pt[:])
            gidx += 1
        nc.scalar.dma_start(
            o_ap[:, c0 : c0 + sz],
            ot[:].rearrange("h (c w) -> h c w", c=sz),
        )
        c0 += sz


def build_module(s_u, s_v):
    nc = bacc.Bacc(
        "TRN2", target_bir_lowering=False, debug=False, num_devices=N_CORES
    )
    x_ap = nc.dram_tensor("x", [H, C, WP], BF16, kind="ExternalInput").ap()
    w_ap = nc.dram_tensor("wts", [H, 4 * H], BF16, kind="ExternalInput").ap()
    o_ap = nc.dram_tensor("out", [H, C, W], BF16, kind="ExternalOutput").ap()
    with tile.TileContext(nc) as tc:
        with ExitStack() as ctx:
            _body(ctx, tc, o_ap, x_ap, w_ap, s_u, s_v)
    nc.compile()
    return nc


def band_mats(k2d):
    """WT[j] = A_j^T where A_j[h, h+i-2] = Kf[i, j] (rows clipped to [0,128))."""
    kf = np.asarray(k2d, np.float32)[::-1, ::-1]
    wts = np.zeros((4, H, H), np.float32)
    for j in range(4):
        for i in range(4):
            d = i - 2  # diagonal offset m - h
            h0, h1 = max(0, -d), min(H, H - d)
            idx = np.arange(h0, h1)
            wts[j, idx + d, idx] = kf[i, j]
    return wts


def _bf16(a):
    import ml_dtypes

    return np.asarray(a).astype(ml_dtypes.bfloat16)


def prep_x(x_core):
    """[C,H,W] f32 -> [H,C,WP] bf16 with zero cols at 0,1 and WP-1."""
    xp = np.zeros((H, x_core.shape[0], WP), np.float32)
    xp[:, :, 2 : 2 + W] = x_core.transpose(1, 0, 2)
    return _bf16(xp)


_module_cache = {}


def kernel(x, kernel, _trace=False, _trace_kwargs=None):
    x = np.asarray(x, np.float32)
    assert x.shape == (B, C, H, W), x.shape
    kf = np.asarray(kernel, np.float32)[::-1, ::-1]
    # column sums of the flipped 2D kernel = flipped 1D W-kernel (separable)
    kw = kf.sum(axis=0) / kf.sum()
    s_u = float(kw[3] / kw[0])  # u = Xp0 + s_u*Xp3 under stationary A_0
    s_v = float(kw[2] / kw[1])  # v = Xp1 + s_v*Xp2 under stationary A_1
    key = (round(s_u, 9), round(s_v, 9))
    if key not in _module_cache:
        _module_cache[key] = build_module(s_u, s_v)
    nc = _module_cache[key]
    wts = _bf16(band_mats(kernel).transpose(1, 0, 2).reshape(H, 4 * H))
    in_maps = [{"x": prep_x(x[i]), "wts": wts.copy()} for i in range(N_CORES)]
    res = run_bass_kernel_spmd(
        nc, in_maps, list(range(N_CORES)), trace=_trace, **(_trace_kwargs or {})
    )
    out = np.stack(
        [
            np.asarray(res.results[i]["out"]).transpose(1, 0, 2).astype(np.float32)
            for i in range(N_CORES)
        ],
        axis=0,
    )
    if _trace:
        return out, res
    return out


# revision 10
# speedup vs baseline: 1.0574x; 1.0574x over previous
"""Depthwise 4x4 blur (upfirdn2d pad=(2,1)) on TRN2, 8 NeuronCores.

Math: out[h,w] = sum_{i,j} Kf[i,j] * x[h+i-2, w+j-2]   (Kf = flipped 2D kernel,
out-of-range terms = zero padding). For each kernel column j this is a banded
128x128 matrix A_j applied over H to a W-shifted slice of the padded image:

    OUT = sum_j A_j @ Xpad[:, j:j+128]      (PSUM accumulation over j)

H-padding is folded into the band clipping of A_j; W-padding is baked into the
host-side layout (stride-131 rows: [0, 0, x0..x127, 0]). Sharding: batch dim
(8 batches -> 8 cores), each core processes 256 images of 128x128.

bf16 everywhere (tolerance is 2e-2; measured rel err ~4e-3): halves HBM
traffic vs f32 (HBM floor ~47 us/core) and streams the PE at 1 col/cycle.
Host-side the block is transposed to [H, C, WP] / [H, C, W] so every DMA is a
dense 2D pattern (multi-KB contiguous run per partition).

v3: the blur is separable, so A_j = kw[j] * Ab. Taps pair up:

    kw0*Ab@Xp0 + kw1*Ab@Xp1 = A_0 @ (Xp0 + (kw1/kw0)*Xp1) = A_0 @ u
    kw3*Ab@Xp3 + kw2*Ab@Xp2 = A_3 @ (Xp3 + (kw2/kw3)*Xp2) = A_3 @ v

u/v are ONE fused DVE op each (scalar_tensor_tensor), so most image groups
need only 2 matmuls instead of 4, moving half the PE work to the otherwise
idle Vector engine. Every MM4_EVERY-th group keeps the plain 4-matmul path to
balance PE (~35us) against DVE (~32us). PSUM->SBUF copies alternate between
Scalar and GpSimd. All engines then sit below the ~47.5us DMA floor.
"""

import numpy as np
from contextlib import ExitStack

import concourse.bass as bass
import concourse.bacc as bacc
import concourse.tile as tile
import concourse.mybir as mybir
from concourse.bass_utils import run_bass_kernel_spmd

N_CORES = 8
B, C, H, W = 8, 256, 128, 128
WP = W + 3         # padded image stride: [0, 0, x0..x127, 0]
GROUP = 4          # images per PSUM bank (4*128 = 512 f32 = one bank)
SUPER = 32         # images per DMA (~8 KB contiguous per partition)
MM4_EVERY = 4      # every Nth group uses the 4-matmul path (PE/DVE balance)

F32 = mybir.dt.float32
BF16 = mybir.dt.bfloat16
MULT = mybir.AluOpType.mult
ADD = mybir.AluOpType.add


def _body(ctx, tc, o_ap, x_ap, w_ap, s_u, s_v):
    nc = tc.nc
    wpool = ctx.enter_context(tc.tile_pool(name="wts", bufs=1))
    xpool = ctx.enter_context(tc.tile_pool(name="xin", bufs=4))
    upool = ctx.enter_context(tc.tile_pool(name="uv", bufs=12))
    opool = ctx.enter_context(tc.tile_pool(name="oup", bufs=4))
    ppool = ctx.enter_context(tc.tile_pool(name="ps", bufs=8, space="PSUM"))

    wt = wpool.tile([H, 4 * H], BF16)

    # ramp-up / ramp-down supertile sizes: small tiles at the ends prime and
    # drain the DMA->prep->matmul->copy->DMA pipeline faster
    sizes = [2, 2, 4, 8, 16] + [SUPER] * ((C - 64) // SUPER) + [16, 8, 4, 2, 2]
    assert sum(sizes) == C

    def emit_prep(peng, xt3, uv3, gi, gc):
        # GpSimd can't touch PSUM and only has plain tensor_tensor, but the
        # SBUF->SBUF prep adds are exactly that (when the kernel is symmetric)
        for k, (ja, jb, s) in enumerate(((0, 3, s_u), (1, 2, s_v))):
            va = xt3[:, gi : gi + gc, ja : ja + W]
            vb = xt3[:, gi : gi + gc, jb : jb + W]
            if s == 1.0:
                peng.tensor_tensor(uv3[:, k], va, vb, ADD)
            else:
                nc.vector.scalar_tensor_tensor(uv3[:, k], vb, s, va, MULT, ADD)

    # first data tile + weights lead on the ACT ring; steady-state input DMA
    # owns the SP ring while outputs go back on ACT
    first = True
    c0 = 0
    for sz in sizes:
        xt = xpool.tile([H, sz * WP], BF16, tag="xt")
        xt3 = xt[:].rearrange("h (c w) -> h c w", c=sz)
        eng = nc.scalar if first else nc.sync
        eng.dma_start(xt3, x_ap[:, c0 : c0 + sz])
        if first:
            nc.scalar.dma_start(wt[:], w_ap)
            first = False
        ot = opool.tile([H, sz * W], BF16, tag="ot")
        groups = [(i * GROUP, min(GROUP, sz - i * GROUP))
                  for i in range((sz + GROUP - 1) // GROUP)]
        ng = len(groups)
        # prep-path groups sit at the END of the PE's program order and their
        # DVE/GPS ops are emitted FIRST (right after the input DMA), so the
        # slower vector engines stay ahead of the PE and never stall it
        # head-of-line. 8-group supertile: 4x mm4-path, 3x DVE, 1x GPS.
        n_prep = {8: 4, 4: 2}.get(ng, 0)
        prep = {}
        for pi in range(n_prep):
            gidx = ng - n_prep + pi
            gi, gc = groups[gidx]
            peng = nc.gpsimd if (ng == 8 and pi == n_prep - 1) else nc.vector
            uv = upool.tile([H, 2 * gc * W], BF16, tag="uv")
            uv3 = uv[:].rearrange("h (k c w) -> h k c w", k=2, c=gc)
            emit_prep(peng, xt3, uv3, gi, gc)
            prep[gidx] = uv3
        for gidx, (gi, gc) in enumerate(groups):
            pt = ppool.tile([H, gc * W], F32, tag="pt")
            if gidx in prep:
                uv3 = prep[gidx]
                nc.tensor.matmul(pt[:], wt[:, :H], uv3[:, 0], start=True, stop=False)
                nc.tensor.matmul(pt[:], wt[:, H : 2 * H], uv3[:, 1], start=False, stop=True)
            else:
                for j in range(4):
                    nc.tensor.matmul(
                        pt[:], wt[:, j * H : (j + 1) * H],
                        xt3[:, gi : gi + gc, j : j + W],
                        start=(j == 0), stop=(j == 3),
                    )
            od = ot[:, gi * W : (gi + gc) * W]
            if ng == 8 and gidx == 5:
                nc.vector.tensor_copy(od, pt[:])
            else:
                nc.scalar.copy(od, pt[:])
        nc.scalar.dma_start(
            o_ap[:, c0 : c0 + sz],
            ot[:].rearrange("h (c w) -> h c w", c=sz),
        )
        c0 += sz


def build_module(s_u, s_v):
    nc = bacc.Bacc(
        "TRN2", target_bir_lowering=False, debug=False, num_devices=N_CORES
    )
    x_ap = nc.dram_tensor("x", [H, C, WP], BF16, kind="ExternalInput").ap()
    w_ap = nc.dram_tensor("wts", [H, 4 * H], BF16, kind="ExternalInput").ap()
    o_ap = nc.dram_tensor("out", [H, C, W], BF16, kind="ExternalOutput").ap()
    with tile.TileContext(nc) as tc:
        with ExitStack() as ctx:
            _body(ctx, tc, o_ap, x_ap, w_ap, s_u, s_v)
    nc.compile()
    return nc


def band_mats(k2d):
    """WT[j] = A_j^T where A_j[h, h+i-2] = Kf[i, j] (rows clipped to [0,128))."""
    kf = np.asarray(k2d, np.float32)[::-1, ::-1]
    wts = np.zeros((4, H, H), np.float32)
    for j in range(4):
        for i in range(4):
            d = i - 2  # diagonal offset m - h
            h0, h1 = max(0, -d), min(H, H - d)
            idx = np.arange(h0, h1)
            wts[j, idx + d, idx] = kf[i, j]
    return wts


def _bf16(a):
    import ml_dtypes

    return np.asarray(a).astype(ml_dtypes.bfloat16)


def prep_x(x_core):
    """[C,H,W] f32 -> [H,C,WP] bf16 with zero cols at 0,1 and WP-1."""
    xp = np.zeros((H, x_core.shape[0], WP), np.float32)
    xp[:, :, 2 : 2 + W] = x_core.transpose(1, 0, 2)
    return _bf16(xp)


_module_cache = {}


def kernel(x, kernel, _trace=False, _trace_kwargs=None):
    x = np.asarray(x, np.float32)
    assert x.shape == (B, C, H, W), x.shape
    kf = np.asarray(kernel, np.float32)[::-1, ::-1]
    # column sums of the flipped 2D kernel = flipped 1D W-kernel (separable)
    kw = kf.sum(axis=0) / kf.sum()
    s_u = float(kw[3] / kw[0])  # u = Xp0 + s_u*Xp3 under stationary A_0
    s_v = float(kw[2] / kw[1])  # v = Xp1 + s_v*Xp2 under stationary A_1
    key = (round(s_u, 9), round(s_v, 9))
    if key not in _module_cache:
        _module_cache[key] = build_module(s_u, s_v)
    nc = _module_cache[key]
    wts = _bf16(band_mats(kernel).transpose(1, 0, 2).reshape(H, 4 * H))
    in_maps = [{"x": prep_x(x[i]), "wts": wts.copy()} for i in range(N_CORES)]
    res = run_bass_kernel_spmd(
        nc, in_maps, list(range(N_CORES)), trace=_trace, **(_trace_kwargs or {})
    )
    out = np.stack(
        [
            np.asarray(res.results[i]["out"]).transpose(1, 0, 2).astype(np.float32)
            for i in range(N_CORES)
        ],
        axis=0,
    )
    if _trace:
        return out, res
    return out
